# revision 11
# baseline (speedup 1.0000x reference)
"""Causal multi-head attention for TRN2, sharded across 8 NeuronCores.

Problem: x[4,2048,1024] -> 16-head causal self-attention (head_dim 64) with
QKV + output projections, fp32.

Sharding: core c -> batch b = c // 2, head-group g = c % 2 (heads g*8..g*8+7).
Per core: Q/K/V projections use the 512 weight columns of its head-group
(column-parallel); attention runs over its 8 heads; the output projection
uses the matching 512 rows of wo (row-parallel), so each core emits a
partial [2048,1024] output and the host sums the two partials per batch.
bo is added on the g==0 cores only (g==1 cores receive zeros).

Device design (per core; S=2048, D=1024, HD=64; matmul operands bf16, all
accumulation fp32 in PSUM):
  - Heads processed as PAIRS: head 2t in SBUF partitions 0:64, head 2t+1
    in 64:128 (QT/KT/AT tiles [128, 4, S]). Off-diagonal score matmuls are
    64-row TILED (tiles T0/T8 via base_partition 0/64): both heads' scores
    run CONCURRENTLY on the PE (measured 2.0x). Mode switches (64<->128)
    cost ~106ns, so tiled scores run in 2-round blocks ([S x4] 64-mode |
    [filler + AV x4] 128-mode) and the 4 short DIAGONAL rounds per
    pair-chunk run un-tiled (128-contraction, zero-padded via the
    double-buffered KDz tile) inside the 128-mode region.
  - All score rounds land in ONE persistent 4-bank PSUM tile S4
    [128,4,512] (subtile deps give per-quarter synchronization). A tiled
    block's 4 banks are consumed by ONE batched exp instruction
    [128,4,512] (amortizes the ~210ns per-instruction PSUM-read latency;
    ACT exp is the chunk-3 co-bottleneck). Diagonal rounds exp per-round
    (valid q-range differs).
  - An all-ones column per head's V block accumulates softmax denominators
    in psum row 64 (AV cost is per-streamed-column, so output-partition
    padding is free). Causal: per k-block only the valid q-range is
    computed; the diagonal 128x128 block is masked by a DVE multiply with
    an upper-triangular tile.
  - Normalization per head: DVE copy of the denominator row,
    reciprocal_approx_fast, gpsimd partition-broadcast, DVE multiply.
  - Startup: weights/x are shipped HOST-PRE-PACKED so every big DMA is
    partition-contiguous (8KB runs, ~380 GB/s; the naive rearranged loads
    have 1KB runs at ~2-3x less). DMAs issue in consumption order; wv/wo
    are chained behind the critical path via marker-copy WAW deps. The
    chunk-0 Q and K projections run as one interleaved d-major wave over 8
    single-bank PSUM slots so the PE tracks DMA arrivals; V follows on 4
    slots. Dummy matmuls bridge the pre-DMA window to warm the HAM
    clock-gate.
  - Pipeline: chunk j's attention interleaves later chunks' projections as
    PE filler, rebalanced so chunk 3 (ACT-bound) gets exactly the 24
    out-projection groups of chunks 0..2: chunk0 <- proj(1)+Q(2),
    chunk1 <- K(2),V(2),Q(3), chunk2 <- K(3),V(3), chunk3 <- outproj x24.
    Chunk 3's own out-projection is the tail, bridged by dummies.
"""

import os
from contextlib import ExitStack

import numpy as np

import concourse.bacc as bacc
import concourse.mybir as mybir
import concourse.tile as tile
from concourse.bass_utils import run_bass_kernel_spmd
from concourse.masks import make_upper_triangular

F32 = mybir.dt.float32
BF16 = mybir.dt.bfloat16
AF = mybir.ActivationFunctionType
ALU = mybir.AluOpType

B = 4
S = 2048
D = 1024
HD = 64
HG = 8  # heads per core
QC = HG * HD  # 512 local q/k/v columns
N_CORES = 8

_NC_CACHE = {}
LAST_RESULT = None  # BassKernelResults of the most recent kernel() call


def _build_nc(s: int = S, num_devices: int = N_CORES):
    P = 128
    NQ = s // 512
    NS = s // P
    ND = D // P
    NT = QC // P  # 4 head pairs
    VW = HD + 1  # 65: per-head V block width (64 cols + ones col)
    VPAD = 7 * VW + P  # 583: last head's lhsT slice must fit

    nc = bacc.Bacc("TRN2", target_bir_lowering=False, debug=False, num_devices=num_devices)

    # All big inputs host-pre-packed to [128, ...] partition-contiguous.
    xT_d = nc.dram_tensor("xTp", [P, ND * s], BF16, kind="ExternalInput").ap()
    wq_d = nc.dram_tensor("wqp", [P, ND * QC], BF16, kind="ExternalInput").ap()
    wk_d = nc.dram_tensor("wkp", [P, ND * QC], BF16, kind="ExternalInput").ap()
    wv_d = nc.dram_tensor("wvp", [P, ND * QC], BF16, kind="ExternalInput").ap()
    wo_d = nc.dram_tensor("wop", [P, NT * D], BF16, kind="ExternalInput").ap()
    bq_d = nc.dram_tensor("bq", [QC], F32, kind="ExternalInput").ap()
    bk_d = nc.dram_tensor("bk", [QC], F32, kind="ExternalInput").ap()
    bv_d = nc.dram_tensor("bv", [QC], F32, kind="ExternalInput").ap()
    bo_d = nc.dram_tensor("bo", [D], F32, kind="ExternalInput").ap()
    out_d = nc.dram_tensor("out", [s, D], F32, kind="ExternalOutput").ap()

    with tile.TileContext(nc) as tc:
        with ExitStack() as ctx:
            consts = ctx.enter_context(tc.tile_pool(name="consts", bufs=1))
            persist = ctx.enter_context(tc.tile_pool(name="persist", bufs=1))
            e2_pool = ctx.enter_context(tc.tile_pool(name="e2pool", bufs=4))
            n_pool = ctx.enter_context(tc.tile_pool(name="npool", bufs=4))
            b_pool = ctx.enter_context(tc.tile_pool(name="bpool", bufs=4))
            o_pool = ctx.enter_context(tc.tile_pool(name="opool", bufs=3))
            proj_psum = ctx.enter_context(tc.tile_pool(name="proj_ps", bufs=2, space="PSUM"))
            s_psum = ctx.enter_context(tc.tile_pool(name="s_ps", bufs=1, space="PSUM"))
            a_psum = ctx.enter_context(tc.tile_pool(name="a_ps", bufs=2, space="PSUM"))

            # ---- dummy-weight tile via memset: warmup needs no DMA/gpsimd ----
            dmy = consts.tile([P, P], BF16)
            nc.vector.memset(dmy[:], 0.0078125)

            def dummy(n=1):
                """Keep-warm matmuls (dmy x dmy into a rotating proj bank)."""
                kw = proj_psum.tile([P, 512], F32, tag="pp", name="kw")
                for _ in range(n):
                    nc.tensor.matmul(
                        kw[:, 0:P], lhsT=dmy[:], rhs=dmy[:], start=True, stop=True
                    )

            # bridge the DMA-load window with continuous PE activity so the
            # HAM clock gate trips to 8/8 and stays there (~214ns each)
            dummy(44)

            tri = consts.tile([P, P], F32)
            make_upper_triangular(nc, tri[:], val=1.0, diag=True)
            tri_b = consts.tile([P, P], BF16)
            nc.any.tensor_copy(tri_b[:], tri[:])

            bqc = consts.tile([P, NT], F32)
            bkc = consts.tile([P, NT], F32)
            bv1 = consts.tile([1, QC], F32)
            bo1 = consts.tile([1, D], F32)
            bvb = consts.tile([P, QC], F32)
            bob = consts.tile([P, D], F32)

            # ---- persistent SBUF tensors ----
            QT = persist.tile([P, NT, s], BF16)
            KT = persist.tile([P, NT, s], BF16)
            # per-chunk diag K, zero-padded; double-buffered by chunk parity
            # (chunk j+1's K filler evac must not collide with chunk j's reads)
            KDz = persist.tile([P, HG, 2, 512], BF16)
            V = persist.tile([P, NS, VPAD + 1], BF16)
            AT = persist.tile([P, NT, s], BF16)
            xT = persist.tile([P, ND, s], BF16)
            wq_sb = persist.tile([P, ND, QC], BF16)
            wk_sb = persist.tile([P, ND, QC], BF16)
            wv_sb = persist.tile([P, ND, QC], BF16)
            wo_sb = persist.tile([P, NT, D], BF16)

            # the single 4-bank score tile (subtile deps sync per quarter)
            S4 = s_psum.tile([P, 4, 512], F32, name="S4")

            # V pad/ones + KDz constant zero halves, on-chip
            nc.any.memset(V[:, :, 7 * VW + HD + 1 :], 0.0)
            nc.any.memset(
                V[:, :, 0 : HG * VW].rearrange("p s (h c) -> p s h c", c=VW)[:, :, :, HD : HD + 1],
                1.0,
            )
            nc.any.memset(
                KDz[64:128].rearrange("p (t two) pr c -> p t two pr c", two=2)[:, :, 0], 0.0
            )
            nc.any.memset(
                KDz[0:64].rearrange("p (t two) pr c -> p t two pr c", two=2)[:, :, 1], 0.0
            )

            # ---- big input DMAs (contiguous), in consumption order ----
            nc.sync.dma_start(wq_sb[:], wq_d)
            nc.sync.dma_start(wk_sb[:], wk_d)
            for dd in range(0, ND, 2):
                nc.sync.dma_start(xT[:, dd : dd + 2, :], xT_d[:, dd * s : (dd + 2) * s])
            # small consts (needed only by evacuations, ~20us in)
            nc.sync.dma_start(bqc[:], bq_d.rearrange("(t p) -> p t", p=P))
            nc.sync.dma_start(bkc[:], bk_d.rearrange("(t p) -> p t", p=P))
            nc.sync.dma_start(bv1[:], bv_d[None, :])
            nc.sync.dma_start(bo1[:], bo_d[None, :])
            nc.gpsimd.partition_broadcast(bvb[:], bv1[0:1, :])
            nc.gpsimd.partition_broadcast(bob[:], bo1[0:1, :])
            # wv/wo deferred via marker-copy WAW deps: they'd otherwise share
            # HBM bandwidth with (and delay) the critical path above.
            nc.vector.tensor_copy(wv_sb[0:1, 0, 0:2], xT[0:1, 6, 0:2])
            nc.sync.dma_start(wv_sb[:, 0:4, :], wv_d[:, 0 : 4 * QC])
            nc.sync.dma_start(wv_sb[:, 4:8, :], wv_d[:, 4 * QC : 8 * QC])
            nc.vector.tensor_copy(wo_sb[0:1, 0, 0:2], wv_sb[0:1, 7, 0:2])
            nc.sync.dma_start(wo_sb[:], wo_d)

            # ---- evacuation helpers ----
            def evac_q(ps, t, j):
                js = slice(j * 512, (j + 1) * 512)
                nc.vector.tensor_scalar_add(QT[:, t, js], ps[:], bqc[:, t : t + 1])

            def evac_k(ps, t, j):
                js = slice(j * 512, (j + 1) * 512)
                nc.vector.tensor_scalar_add(KT[:, t, js], ps[:], bkc[:, t : t + 1])
                # diag copy for chunk j (zero halves are persistent)
                nc.vector.tensor_scalar_add(
                    KDz[0:64, 2 * t, j % 2, :], ps[0:64, :], bkc[0:64, t : t + 1]
                )
                nc.vector.tensor_scalar_add(
                    KDz[64:128, 2 * t + 1, j % 2, :], ps[64:128, :], bkc[64:128, t : t + 1]
                )

            def evac_v(ps, st):
                dst = V[:, st, 0 : HG * VW].rearrange("p (h c) -> p h c", c=VW)[:, :, 0:HD]
                src = ps.rearrange("p (h c) -> p h c", c=HD)
                bsrc = bvb.rearrange("p (h c) -> p h c", c=HD)
                nc.vector.tensor_tensor(dst, src, bsrc, ALU.add)

            # ---- startup: interleaved Q+K chunk-0 wave over 8 banks ----
            qslots = [
                proj_psum.tile([P, 512], F32, tag="pp", name="wv0"),
                proj_psum.tile([P, 512], F32, tag="pp", name="wv1"),
                a_psum.tile([P, 512], F32, tag="a", name="wv2"),
                a_psum.tile([P, 512], F32, tag="a", name="wv3"),
            ]
            for d in range(ND):
                for t in range(NT):
                    nc.tensor.matmul(
                        qslots[t][:],
                        lhsT=wq_sb[:, d, t * P : (t + 1) * P],
                        rhs=xT[:, d, 0:512],
                        start=(d == 0),
                        stop=(d == ND - 1),
                    )
                for t in range(NT):
                    nc.tensor.matmul(
                        S4[:, t, :],
                        lhsT=wk_sb[:, d, t * P : (t + 1) * P],
                        rhs=xT[:, d, 0:512],
                        start=(d == 0),
                        stop=(d == ND - 1),
                        skip_group_check=True,
                    )
            for t in range(NT):
                evac_q(qslots[t], t, 0)
            for t in range(NT):
                evac_k(S4[:, t, :], t, 0)

            # V s-tiles 0..3, d-major over 4 banks
            vslots = [
                proj_psum.tile([P, 512], F32, tag="pp", name="vs0"),
                proj_psum.tile([P, 512], F32, tag="pp", name="vs1"),
                a_psum.tile([P, 512], F32, tag="a", name="vs2"),
                a_psum.tile([P, 512], F32, tag="a", name="vs3"),
            ]
            for d in range(ND):
                for st in range(4):
                    nc.tensor.matmul(
                        vslots[st][:],
                        lhsT=xT[:, d, st * P : (st + 1) * P],
                        rhs=wv_sb[:, d, :],
                        start=(d == 0),
                        stop=(d == ND - 1),
                    )
            for st in range(4):
                evac_v(vslots[st], st)

            # ---- filler units ----
            def proj_group(j, g):
                """One psum-group of the j-chunk projections; g in 0..11."""
                js = slice(j * 512, (j + 1) * 512)
                kind, t = divmod(g, NT)
                ps = proj_psum.tile([P, 512], F32, tag="pp", name="pp")
                if kind == 0:  # Q
                    for d in range(ND):
                        nc.tensor.matmul(
                            ps[:],
                            lhsT=wq_sb[:, d, t * P : (t + 1) * P],
                            rhs=xT[:, d, js],
                            start=(d == 0),
                            stop=(d == ND - 1),
                        )
                    evac_q(ps, t, j)
                elif kind == 1:  # K
                    for d in range(ND):
                        nc.tensor.matmul(
                            ps[:],
                            lhsT=wk_sb[:, d, t * P : (t + 1) * P],
                            rhs=xT[:, d, js],
                            start=(d == 0),
                            stop=(d == ND - 1),
                        )
                    evac_k(ps, t, j)
                else:  # V s-tile 4j+t
                    st = 4 * j + t
                    for d in range(ND):
                        nc.tensor.matmul(
                            ps[:],
                            lhsT=xT[:, d, st * P : (st + 1) * P],
                            rhs=wv_sb[:, d, :],
                            start=(d == 0),
                            stop=(d == ND - 1),
                        )
                    evac_v(ps, st)

            def out_proj_group(j, g):
                st = 4 * j + g // 2
                oc = g % 2
                o_ps = proj_psum.tile([P, 512], F32, tag="pp", name="o_ps")
                for t2 in range(NT):
                    nc.tensor.matmul(
                        o_ps[:],
                        lhsT=AT[:, t2, st * P : (st + 1) * P],
                        rhs=wo_sb[:, t2, oc * 512 : (oc + 1) * 512],
                        start=(t2 == 0),
                        stop=(t2 == NT - 1),
                    )
                ot = o_pool.tile([P, 512], F32, name="ot")
                nc.vector.tensor_tensor(
                    ot[:], o_ps[:], bob[:, oc * 512 : (oc + 1) * 512], ALU.add
                )
                nc.sync.dma_start(
                    out_d[st * P : (st + 1) * P, oc * 512 : (oc + 1) * 512], ot[:]
                )

            # ---- attention pair-chunk ----
            def attn_pair(j, t, filler, f_lo, f_hi):
                """Heads (2t, 2t+1) on q-chunk j. filler[f_lo:f_hi] emitted in
                128-mode regions: one slot per tiled block + 2 diag slots."""
                nkb = 4 * j + 4
                ntb = 2 * j  # tiled (off-diagonal) 2-round blocks
                nslot = ntb + 3  # 1 leading + per-block + 2 diagonal
                A0 = a_psum.tile([P, 512], F32, tag="a", name="A0")
                A1 = a_psum.tile([P, 512], F32, tag="a", name="A1")
                jq = j * 512
                nfill = f_hi - f_lo
                slot = 0

                def fill_slot():
                    nonlocal slot
                    k0 = f_lo + (nfill * slot) // nslot
                    k1 = f_lo + (nfill * (slot + 1)) // nslot
                    for f in filler[k0:k1]:
                        f()
                    slot += 1

                def av_round(r, y0, Erhs0, Erhs1):
                    nc.tensor.matmul(
                        A0[:, y0:],
                        lhsT=V[:, r, (2 * t) * VW : (2 * t) * VW + P],
                        rhs=Erhs0,
                        start=(r == 0),
                        stop=(r == nkb - 1),
                    )
                    nc.tensor.matmul(
                        A1[:, y0:],
                        lhsT=V[:, r, (2 * t + 1) * VW : (2 * t + 1) * VW + P],
                        rhs=Erhs1,
                        start=(r == 0),
                        stop=(r == nkb - 1),
                    )

                # leading slot covers the previous pair's exp/norm tail
                fill_slot()

                # off-diagonal: 64-row-tiled score pairs, 2 rounds per
                # block, one 2-bank batched exp per round
                for blk in range(ntb):
                    rr = (2 * blk, 2 * blk + 1)
                    Es = []
                    for q2, r in enumerate(rr):
                        nc.tensor.matmul(
                            S4[:, 2 * q2, :],
                            lhsT=KT[0:64, t, r * P : (r + 1) * P],
                            rhs=QT[0:64, t, jq : jq + 512],
                            start=True,
                            stop=True,
                        )
                        nc.tensor.matmul(
                            S4[:, 2 * q2 + 1, :],
                            lhsT=KT[64:128, t, r * P : (r + 1) * P],
                            rhs=QT[64:128, t, jq : jq + 512],
                            start=True,
                            stop=True,
                        )
                        E2 = e2_pool.tile([P, 2, 512], BF16, tag="e", name="E2")
                        nc.scalar.activation(
                            E2[:], S4[:, 2 * q2 : 2 * q2 + 2, :], AF.Exp, scale=0.125
                        )
                        Es.append((r, E2))
                    fill_slot()
                    for r, E2 in Es:
                        av_round(r, 0, E2[:, 0, :], E2[:, 1, :])

                # diagonal region: 4 un-tiled (128-contraction) rounds using
                # S4 quarter-pairs, per-round exp (valid q-range differs)
                for half in range(2):
                    Es = []
                    for q2 in range(2):
                        r = 4 * j + 2 * half + q2
                        y0 = P * (r - 4 * j)
                        for hh in range(2):
                            nc.tensor.matmul(
                                S4[:, 2 * q2 + hh, y0:],
                                lhsT=KDz[:, 2 * t + hh, j % 2, y0 : y0 + P],
                                rhs=QT[:, t, jq + y0 : jq + 512],
                                start=True,
                                stop=True,
                            )
                        E2 = e2_pool.tile([P, 2, 512], BF16, tag="e", name="E2")
                        nc.scalar.activation(
                            E2[:, :, y0:],
                            S4[:, 2 * q2 : 2 * q2 + 2, y0:],
                            AF.Exp,
                            scale=0.125,
                        )
                        for i in range(2):  # causal mask on the diag block
                            nc.vector.tensor_tensor(
                                E2[:, i, y0 : y0 + P],
                                E2[:, i, y0 : y0 + P],
                                tri_b[:],
                                ALU.mult,
                            )
                        Es.append((r, y0, E2))
                    fill_slot()
                    for r, y0, E2 in Es:
                        av_round(r, y0, E2[:, 0, y0:], E2[:, 1, y0:])

                # softmax normalization for both heads
                for i, A in enumerate((A0, A1)):
                    sums = n_pool.tile([1, 512], F32, tag="sums", name="sums")
                    nc.vector.tensor_copy(sums[:], A[HD : HD + 1, :])
                    rec = n_pool.tile([1, 512], F32, tag="rec", name="rec")
                    nc.vector.reciprocal_approx_fast(rec[:], sums[:])
                    bc = b_pool.tile([HD, 512], F32, name="bc")
                    nc.gpsimd.partition_broadcast(bc[:], rec[0:1, :])
                    nc.vector.tensor_tensor(
                        AT[64 * i : 64 * i + HD, t, jq : jq + 512],
                        A[0:HD, :],
                        bc[:],
                        ALU.mult,
                    )

            # ---- main pipeline; filler rebalanced toward chunk 3 ----
            def P_(jj, g):
                return lambda: proj_group(jj, g)

            def O_(jj, g):
                return lambda: out_proj_group(jj, g)

            fillers = [
                [P_(1, g) for g in range(12)] + [P_(2, g) for g in range(4)],
                [P_(2, g) for g in range(4, 12)] + [P_(3, g) for g in range(4)],
                [P_(3, g) for g in range(4, 12)],
                [O_(jo, g) for jo in range(NQ - 1) for g in range(8)],
            ]
            for j in range(NQ):
                filler = fillers[j]
                nf = len(filler)
                for t in range(NT):
                    attn_pair(j, t, filler, (nf * t) // NT, (nf * (t + 1)) // NT)

            # tail: bridge the last normalization, then chunk-3 out-proj
            dummy(24)
            for g in range(8):
                out_proj_group(NQ - 1, g)

    nc.compile()

    return nc


def _get_nc():
    if "nc" not in _NC_CACHE:
        _NC_CACHE["nc"] = _build_nc()
    return _NC_CACHE["nc"]


def _pack(w, nd=8):
    """[nd*128, C] -> [128, nd*C] partition-contiguous."""
    ndp, c = w.shape
    p = ndp // nd
    return np.ascontiguousarray(w.reshape(nd, p, c).transpose(1, 0, 2).reshape(p, nd * c))


def make_in_maps(x, wq, bq, wk, bk, wv, bv, wo, bo, n_cores=N_CORES):
    import ml_dtypes

    bf = ml_dtypes.bfloat16
    x = np.asarray(x, np.float32).astype(bf)
    wq, wk, wv, wo = (np.asarray(a, np.float32).astype(bf) for a in (wq, wk, wv, wo))
    bq, bk, bv, bo = (np.asarray(a, np.float32) for a in (bq, bk, bv, bo))
    in_maps = []
    for c in range(n_cores):
        b, g = c // 2, c % 2
        cs = slice(g * QC, (g + 1) * QC)
        in_maps.append(
            {
                "xTp": _pack(np.ascontiguousarray(x[b].T)),
                "wqp": _pack(np.ascontiguousarray(wq[:, cs])),
                "wkp": _pack(np.ascontiguousarray(wk[:, cs])),
                "wvp": _pack(np.ascontiguousarray(wv[:, cs])),
                "wop": _pack(np.ascontiguousarray(wo[cs, :]), nd=4),
                "bq": np.ascontiguousarray(bq[cs]),
                "bk": np.ascontiguousarray(bk[cs]),
                "bv": np.ascontiguousarray(bv[cs]),
                "bo": bo if g == 0 else np.zeros_like(bo),
            }
        )
    return in_maps


def kernel(x, wq, bq, wk, bk, wv, bv, wo, bo):
    global LAST_RESULT
    in_maps = make_in_maps(x, wq, bq, wk, bk, wv, bv, wo, bo)
    nc = _get_nc()
    trace = os.environ.get("MHA_TRACE", "0") == "1"
    res = run_bass_kernel_spmd(nc, in_maps, core_ids=list(range(N_CORES)), trace=trace)
    LAST_RESULT = res

    out = np.empty((B, S, D), np.float32)
    for b in range(B):
        out[b] = res.results[2 * b]["out"] + res.results[2 * b + 1]["out"]
    return out


# revision 12
# speedup vs baseline: 1.3305x; 1.3305x over previous
"""Causal multi-head attention for TRN2, sharded across 8 NeuronCores.

Problem: x[4,2048,1024] -> 16-head causal self-attention (head_dim 64) with
QKV + output projections, fp32.

Sharding: core c -> batch b = c // 2, head-group g = c % 2 (heads g*8..g*8+7).
Per core: Q/K/V projections use the 512 weight columns of its head-group
(column-parallel); attention runs over its 8 heads; the output projection
uses the matching 512 rows of wo (row-parallel), so each core emits a
partial [2048,1024] output and the host sums the two partials per batch.
bo is added on the g==0 cores only (g==1 cores receive zeros).

Device design (per core; S=2048, D=1024, HD=64; matmul operands bf16, all
accumulation fp32 in PSUM):
  - Heads processed as PAIRS: head 2t in SBUF partitions 0:64, head 2t+1
    in 64:128 (QT/KT/AT tiles [128, 4, S]). Off-diagonal score matmuls are
    64-row TILED (tiles T0/T8 via base_partition 0/64): both heads' scores
    run CONCURRENTLY on the PE (measured 2.0x). Mode switches (64<->128)
    cost ~106ns, so tiled scores run in 2-round blocks ([S x4] 64-mode |
    [filler + AV x4] 128-mode) and the 4 short DIAGONAL rounds per
    pair-chunk run un-tiled (128-contraction, zero-padded via the
    double-buffered KDz tile) inside the 128-mode region.
  - All score rounds land in ONE persistent 4-bank PSUM tile S4
    [128,4,512] (subtile deps give per-quarter synchronization). A tiled
    block's 4 banks are consumed by ONE batched exp instruction
    [128,4,512] (amortizes the ~210ns per-instruction PSUM-read latency;
    ACT exp is the chunk-3 co-bottleneck). Diagonal rounds exp per-round
    (valid q-range differs).
  - An all-ones column per head's V block accumulates softmax denominators
    in psum row 64 (AV cost is per-streamed-column, so output-partition
    padding is free). Causal: per k-block only the valid q-range is
    computed; the diagonal 128x128 block is masked by a DVE multiply with
    an upper-triangular tile.
  - Normalization per head: DVE copy of the denominator row,
    reciprocal_approx_fast, gpsimd partition-broadcast, DVE multiply.
  - Startup: weights/x are shipped HOST-PRE-PACKED so every big DMA is
    partition-contiguous (8KB runs, ~380 GB/s; the naive rearranged loads
    have 1KB runs at ~2-3x less). DMAs issue in consumption order; wv/wo
    are chained behind the critical path via marker-copy WAW deps. The
    chunk-0 Q and K projections run as one interleaved d-major wave over 8
    single-bank PSUM slots so the PE tracks DMA arrivals; V follows on 4
    slots. Dummy matmuls bridge the pre-DMA window to warm the HAM
    clock-gate.
  - Pipeline: chunk j's attention interleaves later chunks' projections as
    PE filler, rebalanced so chunk 3 (ACT-bound) gets exactly the 24
    out-projection groups of chunks 0..2: chunk0 <- proj(1)+Q(2),
    chunk1 <- K(2),V(2),Q(3), chunk2 <- K(3),V(3), chunk3 <- outproj x24.
    Chunk 3's own out-projection is the tail, bridged by dummies.
"""

import os
from contextlib import ExitStack

import numpy as np

import concourse.bacc as bacc
import concourse.mybir as mybir
import concourse.tile as tile
from concourse.bass_utils import run_bass_kernel_spmd
from concourse.masks import make_upper_triangular

F32 = mybir.dt.float32
BF16 = mybir.dt.bfloat16
AF = mybir.ActivationFunctionType
ALU = mybir.AluOpType

B = 4
S = 2048
D = 1024
HD = 64
HG = 8  # heads per core
QC = HG * HD  # 512 local q/k/v columns
N_CORES = 8

_NC_CACHE = {}
LAST_RESULT = None  # BassKernelResults of the most recent kernel() call


def _build_nc(s: int = S, num_devices: int = N_CORES):
    P = 128
    NQ = s // 512
    NS = s // P
    ND = D // P
    NT = QC // P  # 4 head pairs
    VW = HD + 1  # 65: per-head V block width (64 cols + ones col)
    VPAD = 7 * VW + P  # 583: last head's lhsT slice must fit

    nc = bacc.Bacc("TRN2", target_bir_lowering=False, debug=False, num_devices=num_devices)

    # All big inputs host-pre-packed to [128, ...] partition-contiguous.
    xT_d = nc.dram_tensor("xTp", [P, ND * s], BF16, kind="ExternalInput").ap()
    wq_d = nc.dram_tensor("wqp", [P, ND * QC], BF16, kind="ExternalInput").ap()
    wk_d = nc.dram_tensor("wkp", [P, ND * QC], BF16, kind="ExternalInput").ap()
    wv_d = nc.dram_tensor("wvp", [P, ND * QC], BF16, kind="ExternalInput").ap()
    wo_d = nc.dram_tensor("wop", [P, NT * D], BF16, kind="ExternalInput").ap()
    bq_d = nc.dram_tensor("bq", [QC], F32, kind="ExternalInput").ap()
    bk_d = nc.dram_tensor("bk", [QC], F32, kind="ExternalInput").ap()
    bv_d = nc.dram_tensor("bv", [QC], F32, kind="ExternalInput").ap()
    bo_d = nc.dram_tensor("bo", [D], F32, kind="ExternalInput").ap()
    out_d = nc.dram_tensor("out", [s, D], F32, kind="ExternalOutput").ap()

    with tile.TileContext(nc) as tc:
        with ExitStack() as ctx:
            consts = ctx.enter_context(tc.tile_pool(name="consts", bufs=1))
            persist = ctx.enter_context(tc.tile_pool(name="persist", bufs=1))
            e2_pool = ctx.enter_context(tc.tile_pool(name="e2pool", bufs=4))
            n_pool = ctx.enter_context(tc.tile_pool(name="npool", bufs=4))
            b_pool = ctx.enter_context(tc.tile_pool(name="bpool", bufs=4))
            o_pool = ctx.enter_context(tc.tile_pool(name="opool", bufs=3))
            proj_psum = ctx.enter_context(tc.tile_pool(name="proj_ps", bufs=2, space="PSUM"))
            s_psum = ctx.enter_context(tc.tile_pool(name="s_ps", bufs=2, space="PSUM"))
            a_psum = ctx.enter_context(tc.tile_pool(name="a_ps", bufs=2, space="PSUM"))

            # ---- dummy-weight tile via memset: warmup needs no DMA/gpsimd ----
            dmy = consts.tile([P, P], BF16)
            nc.vector.memset(dmy[:], 0.0078125)

            def dummy(n=1):
                """Keep-warm matmuls (dmy x dmy into a rotating proj bank)."""
                kw = proj_psum.tile([P, 512], F32, tag="pp", name="kw")
                for _ in range(n):
                    nc.tensor.matmul(
                        kw[:, 0:P], lhsT=dmy[:], rhs=dmy[:], start=True, stop=True
                    )

            # bridge the DMA-load window with continuous PE activity so the
            # HAM clock gate trips to 8/8 and stays there (~214ns each)
            dummy(56)

            tri = consts.tile([P, P], F32)
            make_upper_triangular(nc, tri[:], val=1.0, diag=True)
            tri_b = consts.tile([P, P], BF16)
            nc.any.tensor_copy(tri_b[:], tri[:])

            bqc = consts.tile([P, NT], F32)
            bkc = consts.tile([P, NT], F32)
            bv1 = consts.tile([1, QC], F32)
            bo1 = consts.tile([1, D], F32)
            bvb = consts.tile([P, QC], F32)
            bob = consts.tile([P, D], F32)

            # ---- persistent SBUF tensors ----
            QT = persist.tile([P, NT, s], BF16)
            KT = persist.tile([P, NT, s], BF16)
            # per-chunk diag K, zero-padded; double-buffered by chunk parity
            # (chunk j+1's K filler evac must not collide with chunk j's reads)
            KDz = persist.tile([P, HG, 2, 512], BF16)
            V = persist.tile([P, NS, VPAD + 1], BF16)
            AT = persist.tile([P, NT, s], BF16)
            xT = persist.tile([P, ND, s], BF16)
            wq_sb = persist.tile([P, ND, QC], BF16)
            wk_sb = persist.tile([P, ND, QC], BF16)
            wv_sb = persist.tile([P, ND, QC], BF16)
            wo_sb = persist.tile([P, NT, D], BF16)

            # V pad/ones + KDz constant zero halves, on-chip
            nc.any.memset(V[:, :, 7 * VW + HD + 1 :], 0.0)
            nc.any.memset(
                V[:, :, 0 : HG * VW].rearrange("p s (h c) -> p s h c", c=VW)[:, :, :, HD : HD + 1],
                1.0,
            )
            nc.any.memset(
                KDz[64:128].rearrange("p (t two) pr c -> p t two pr c", two=2)[:, :, 0], 0.0
            )
            nc.any.memset(
                KDz[0:64].rearrange("p (t two) pr c -> p t two pr c", two=2)[:, :, 1], 0.0
            )

            # ---- big input DMAs (contiguous), in consumption order ----
            nc.sync.dma_start(wq_sb[:], wq_d)
            nc.sync.dma_start(wk_sb[:], wk_d)
            for dd in range(0, ND, 2):
                nc.sync.dma_start(xT[:, dd : dd + 2, :], xT_d[:, dd * s : (dd + 2) * s])
            # small consts (needed only by evacuations, ~20us in)
            nc.sync.dma_start(bqc[:], bq_d.rearrange("(t p) -> p t", p=P))
            nc.sync.dma_start(bkc[:], bk_d.rearrange("(t p) -> p t", p=P))
            nc.sync.dma_start(bv1[:], bv_d[None, :])
            nc.sync.dma_start(bo1[:], bo_d[None, :])
            nc.gpsimd.partition_broadcast(bvb[:], bv1[0:1, :])
            nc.gpsimd.partition_broadcast(bob[:], bo1[0:1, :])
            # wv/wo deferred via marker-copy WAW deps: they'd otherwise share
            # HBM bandwidth with (and delay) the critical path above.
            nc.vector.tensor_copy(wv_sb[0:1, 0, 0:2], xT[0:1, 6, 0:2])
            nc.sync.dma_start(wv_sb[:, 0:4, :], wv_d[:, 0 : 4 * QC])
            nc.sync.dma_start(wv_sb[:, 4:8, :], wv_d[:, 4 * QC : 8 * QC])
            nc.vector.tensor_copy(wo_sb[0:1, 0, 0:2], wv_sb[0:1, 7, 0:2])
            nc.sync.dma_start(wo_sb[:], wo_d)

            # ---- evacuation helpers ----
            def evac_q(ps, t, j):
                js = slice(j * 512, (j + 1) * 512)
                nc.vector.tensor_scalar_add(QT[:, t, js], ps[:], bqc[:, t : t + 1])

            def evac_k(ps, t, j):
                js = slice(j * 512, (j + 1) * 512)
                nc.vector.tensor_scalar_add(KT[:, t, js], ps[:], bkc[:, t : t + 1])
                # diag copy for chunk j (zero halves are persistent)
                nc.vector.tensor_scalar_add(
                    KDz[0:64, 2 * t, j % 2, :], ps[0:64, :], bkc[0:64, t : t + 1]
                )
                nc.vector.tensor_scalar_add(
                    KDz[64:128, 2 * t + 1, j % 2, :], ps[64:128, :], bkc[64:128, t : t + 1]
                )

            def evac_v(ps, st):
                dst = V[:, st, 0 : HG * VW].rearrange("p (h c) -> p h c", c=VW)[:, :, 0:HD]
                src = ps.rearrange("p (h c) -> p h c", c=HD)
                bsrc = bvb.rearrange("p (h c) -> p h c", c=HD)
                nc.vector.tensor_tensor(dst, src, bsrc, ALU.add)

            # ---- startup: interleaved Q+K chunk-0 wave over 8 banks ----
            qslots = [
                proj_psum.tile([P, 512], F32, tag="pp", name="wv0"),
                proj_psum.tile([P, 512], F32, tag="pp", name="wv1"),
                a_psum.tile([P, 512], F32, tag="a", name="wv2"),
                a_psum.tile([P, 512], F32, tag="a", name="wv3"),
            ]
            ks0 = s_psum.tile([P, 2, 512], F32, tag="s", name="ks0")
            ks1 = s_psum.tile([P, 2, 512], F32, tag="s", name="ks1")
            kslots = [ks0[:, 0, :], ks0[:, 1, :], ks1[:, 0, :], ks1[:, 1, :]]
            for d in range(ND):
                for t in range(NT):
                    nc.tensor.matmul(
                        qslots[t][:],
                        lhsT=wq_sb[:, d, t * P : (t + 1) * P],
                        rhs=xT[:, d, 0:512],
                        start=(d == 0),
                        stop=(d == ND - 1),
                    )
                for t in range(NT):
                    nc.tensor.matmul(
                        kslots[t],
                        lhsT=wk_sb[:, d, t * P : (t + 1) * P],
                        rhs=xT[:, d, 0:512],
                        start=(d == 0),
                        stop=(d == ND - 1),
                        skip_group_check=True,
                    )
            for t in range(NT):
                evac_q(qslots[t], t, 0)
            for t in range(NT):
                evac_k(kslots[t], t, 0)

            # V s-tiles 0..3, d-major over 4 banks
            vslots = [
                proj_psum.tile([P, 512], F32, tag="pp", name="vs0"),
                proj_psum.tile([P, 512], F32, tag="pp", name="vs1"),
                a_psum.tile([P, 512], F32, tag="a", name="vs2"),
                a_psum.tile([P, 512], F32, tag="a", name="vs3"),
            ]
            for d in range(ND):
                for st in range(4):
                    nc.tensor.matmul(
                        vslots[st][:],
                        lhsT=xT[:, d, st * P : (st + 1) * P],
                        rhs=wv_sb[:, d, :],
                        start=(d == 0),
                        stop=(d == ND - 1),
                    )
            for st in range(4):
                evac_v(vslots[st], st)

            # ---- filler units ----
            def proj_group(j, g):
                """One psum-group of the j-chunk projections; g in 0..11."""
                js = slice(j * 512, (j + 1) * 512)
                kind, t = divmod(g, NT)
                ps = proj_psum.tile([P, 512], F32, tag="pp", name="pp")
                if kind == 0:  # Q
                    for d in range(ND):
                        nc.tensor.matmul(
                            ps[:],
                            lhsT=wq_sb[:, d, t * P : (t + 1) * P],
                            rhs=xT[:, d, js],
                            start=(d == 0),
                            stop=(d == ND - 1),
                        )
                    evac_q(ps, t, j)
                elif kind == 1:  # K
                    for d in range(ND):
                        nc.tensor.matmul(
                            ps[:],
                            lhsT=wk_sb[:, d, t * P : (t + 1) * P],
                            rhs=xT[:, d, js],
                            start=(d == 0),
                            stop=(d == ND - 1),
                        )
                    evac_k(ps, t, j)
                else:  # V s-tile 4j+t
                    st = 4 * j + t
                    for d in range(ND):
                        nc.tensor.matmul(
                            ps[:],
                            lhsT=xT[:, d, st * P : (st + 1) * P],
                            rhs=wv_sb[:, d, :],
                            start=(d == 0),
                            stop=(d == ND - 1),
                        )
                    evac_v(ps, st)

            def out_proj_group(j, g):
                st = 4 * j + g // 2
                oc = g % 2
                o_ps = proj_psum.tile([P, 512], F32, tag="pp", name="o_ps")
                for t2 in range(NT):
                    nc.tensor.matmul(
                        o_ps[:],
                        lhsT=AT[:, t2, st * P : (st + 1) * P],
                        rhs=wo_sb[:, t2, oc * 512 : (oc + 1) * 512],
                        start=(t2 == 0),
                        stop=(t2 == NT - 1),
                    )
                ot = o_pool.tile([P, 512], F32, name="ot")
                nc.vector.tensor_tensor(
                    ot[:], o_ps[:], bob[:, oc * 512 : (oc + 1) * 512], ALU.add
                )
                nc.sync.dma_start(
                    out_d[st * P : (st + 1) * P, oc * 512 : (oc + 1) * 512], ot[:]
                )

            # ---- attention pair-chunk ----
            def attn_pair(j, t, filler, f_lo, f_hi):
                """Heads (2t, 2t+1) on q-chunk j. filler[f_lo:f_hi] emitted in
                128-mode regions: one slot per tiled block + 2 diag slots."""
                nkb = 4 * j + 4
                ntb = 2 * j  # tiled (off-diagonal) 2-round blocks
                nslot = ntb + 3  # 1 leading + per-block + 2 diagonal
                A0 = a_psum.tile([P, 512], F32, tag="a", name="A0")
                A1 = a_psum.tile([P, 512], F32, tag="a", name="A1")
                jq = j * 512
                nfill = f_hi - f_lo
                slot = 0

                def fill_slot():
                    nonlocal slot
                    k0 = f_lo + (nfill * slot) // nslot
                    k1 = f_lo + (nfill * (slot + 1)) // nslot
                    for f in filler[k0:k1]:
                        f()
                    slot += 1

                def av_round(r, y0, Erhs0, Erhs1):
                    nc.tensor.matmul(
                        A0[:, y0:],
                        lhsT=V[:, r, (2 * t) * VW : (2 * t) * VW + P],
                        rhs=Erhs0,
                        start=(r == 0),
                        stop=(r == nkb - 1),
                    )
                    nc.tensor.matmul(
                        A1[:, y0:],
                        lhsT=V[:, r, (2 * t + 1) * VW : (2 * t + 1) * VW + P],
                        rhs=Erhs1,
                        start=(r == 0),
                        stop=(r == nkb - 1),
                    )

                # leading slot covers the previous pair's exp/norm tail
                fill_slot()

                # off-diagonal: 64-row-tiled score pairs, 2 rounds per
                # block, one 2-bank batched exp per round
                for blk in range(ntb):
                    rr = (2 * blk, 2 * blk + 1)
                    Es = []
                    for r in rr:
                        S2 = s_psum.tile([P, 2, 512], F32, tag="s", name="S2")
                        nc.tensor.matmul(
                            S2[:, 0, :],
                            lhsT=KT[0:64, t, r * P : (r + 1) * P],
                            rhs=QT[0:64, t, jq : jq + 512],
                            start=True,
                            stop=True,
                        )
                        nc.tensor.matmul(
                            S2[:, 1, :],
                            lhsT=KT[64:128, t, r * P : (r + 1) * P],
                            rhs=QT[64:128, t, jq : jq + 512],
                            start=True,
                            stop=True,
                        )
                        E2 = e2_pool.tile([P, 2, 512], BF16, tag="e", name="E2")
                        nc.scalar.activation(E2[:], S2[:], AF.Exp, scale=0.125)
                        Es.append((r, E2))
                    fill_slot()
                    for r, E2 in Es:
                        av_round(r, 0, E2[:, 0, :], E2[:, 1, :])

                # diagonal region: 4 un-tiled (128-contraction) rounds using
                # S4 quarter-pairs, per-round exp (valid q-range differs)
                for half in range(2):
                    Es = []
                    for q2 in range(2):
                        r = 4 * j + 2 * half + q2
                        y0 = P * (r - 4 * j)
                        S2 = s_psum.tile([P, 2, 512], F32, tag="s", name="S2d")
                        for hh in range(2):
                            nc.tensor.matmul(
                                S2[:, hh, y0:],
                                lhsT=KDz[:, 2 * t + hh, j % 2, y0 : y0 + P],
                                rhs=QT[:, t, jq + y0 : jq + 512],
                                start=True,
                                stop=True,
                            )
                        E2 = e2_pool.tile([P, 2, 512], BF16, tag="e", name="E2")
                        nc.scalar.activation(
                            E2[:, :, y0:],
                            S2[:, :, y0:],
                            AF.Exp,
                            scale=0.125,
                        )
                        for i in range(2):  # causal mask on the diag block
                            nc.vector.tensor_tensor(
                                E2[:, i, y0 : y0 + P],
                                E2[:, i, y0 : y0 + P],
                                tri_b[:],
                                ALU.mult,
                            )
                        Es.append((r, y0, E2))
                    fill_slot()
                    for r, y0, E2 in Es:
                        av_round(r, y0, E2[:, 0, y0:], E2[:, 1, y0:])

                # softmax normalization for both heads
                for i, A in enumerate((A0, A1)):
                    sums = n_pool.tile([1, 512], F32, tag="sums", name="sums")
                    nc.vector.tensor_copy(sums[:], A[HD : HD + 1, :])
                    rec = n_pool.tile([1, 512], F32, tag="rec", name="rec")
                    nc.vector.reciprocal_approx_fast(rec[:], sums[:])
                    bc = b_pool.tile([HD, 512], F32, name="bc")
                    nc.gpsimd.partition_broadcast(bc[:], rec[0:1, :])
                    nc.vector.tensor_tensor(
                        AT[64 * i : 64 * i + HD, t, jq : jq + 512],
                        A[0:HD, :],
                        bc[:],
                        ALU.mult,
                    )

            # ---- main pipeline; filler rebalanced toward chunk 3 ----
            def P_(jj, g):
                return lambda: proj_group(jj, g)

            def O_(jj, g):
                return lambda: out_proj_group(jj, g)

            fillers = [
                [P_(1, g) for g in range(12)] + [P_(2, g) for g in range(4)],
                [P_(2, g) for g in range(4, 12)] + [P_(3, g) for g in range(4)],
                [P_(3, g) for g in range(4, 12)],
                [O_(jo, g) for jo in range(NQ - 1) for g in range(8)],
            ]
            for j in range(NQ):
                filler = fillers[j]
                nf = len(filler)
                for t in range(NT):
                    attn_pair(j, t, filler, (nf * t) // NT, (nf * (t + 1)) // NT)

            # tail: bridge the last normalization, then chunk-3 out-proj
            dummy(24)
            for g in range(8):
                out_proj_group(NQ - 1, g)

    nc.compile()

    return nc


def _get_nc():
    if "nc" not in _NC_CACHE:
        _NC_CACHE["nc"] = _build_nc()
    return _NC_CACHE["nc"]


def _pack(w, nd=8):
    """[nd*128, C] -> [128, nd*C] partition-contiguous."""
    ndp, c = w.shape
    p = ndp // nd
    return np.ascontiguousarray(w.reshape(nd, p, c).transpose(1, 0, 2).reshape(p, nd * c))


def make_in_maps(x, wq, bq, wk, bk, wv, bv, wo, bo, n_cores=N_CORES):
    import ml_dtypes

    bf = ml_dtypes.bfloat16
    x = np.asarray(x, np.float32).astype(bf)
    wq, wk, wv, wo = (np.asarray(a, np.float32).astype(bf) for a in (wq, wk, wv, wo))
    bq, bk, bv, bo = (np.asarray(a, np.float32) for a in (bq, bk, bv, bo))
    in_maps = []
    for c in range(n_cores):
        b, g = c // 2, c % 2
        cs = slice(g * QC, (g + 1) * QC)
        in_maps.append(
            {
                "xTp": _pack(np.ascontiguousarray(x[b].T)),
                "wqp": _pack(np.ascontiguousarray(wq[:, cs])),
                "wkp": _pack(np.ascontiguousarray(wk[:, cs])),
                "wvp": _pack(np.ascontiguousarray(wv[:, cs])),
                "wop": _pack(np.ascontiguousarray(wo[cs, :]), nd=4),
                "bq": np.ascontiguousarray(bq[cs]),
                "bk": np.ascontiguousarray(bk[cs]),
                "bv": np.ascontiguousarray(bv[cs]),
                "bo": bo if g == 0 else np.zeros_like(bo),
            }
        )
    return in_maps


def kernel(x, wq, bq, wk, bk, wv, bv, wo, bo):
    global LAST_RESULT
    in_maps = make_in_maps(x, wq, bq, wk, bk, wv, bv, wo, bo)
    nc = _get_nc()
    trace = os.environ.get("MHA_TRACE", "0") == "1"
    res = run_bass_kernel_spmd(nc, in_maps, core_ids=list(range(N_CORES)), trace=trace)
    LAST_RESULT = res

    out = np.empty((B, S, D), np.float32)
    for b in range(B):
        out[b] = res.results[2 * b]["out"] + res.results[2 * b + 1]["out"]
    return out


# revision 13
# speedup vs baseline: 1.3608x; 1.0228x over previous
"""Causal multi-head attention for TRN2, sharded across 8 NeuronCores.

Problem: x[4,2048,1024] -> 16-head causal self-attention (head_dim 64) with
QKV + output projections, fp32.

Sharding: core c -> batch b = c // 2, head-group g = c % 2 (heads g*8..g*8+7).
Per core: Q/K/V projections use the 512 weight columns of its head-group
(column-parallel); attention runs over its 8 heads; the output projection
uses the matching 512 rows of wo (row-parallel), so each core emits a
partial [2048,1024] output and the host sums the two partials per batch.
bo is added on the g==0 cores only (g==1 cores receive zeros).

Device design (per core; S=2048, D=1024, HD=64; matmul operands bf16, all
accumulation fp32 in PSUM):
  - Heads processed as PAIRS: head 2t in SBUF partitions 0:64, head 2t+1
    in 64:128 (QT/KT/AT tiles [128, 4, S]). Off-diagonal score matmuls are
    64-row TILED (tiles T0/T8 via base_partition 0/64): both heads' scores
    run CONCURRENTLY on the PE (measured 2.0x). Mode switches (64<->128)
    cost ~106ns, so tiled scores run in 2-round blocks ([S x4] 64-mode |
    [filler + AV x4] 128-mode) and the 4 short DIAGONAL rounds per
    pair-chunk run un-tiled (128-contraction, zero-padded via the
    double-buffered KDz tile) inside the 128-mode region.
  - All score rounds land in ONE persistent 4-bank PSUM tile S4
    [128,4,512] (subtile deps give per-quarter synchronization). A tiled
    block's 4 banks are consumed by ONE batched exp instruction
    [128,4,512] (amortizes the ~210ns per-instruction PSUM-read latency;
    ACT exp is the chunk-3 co-bottleneck). Diagonal rounds exp per-round
    (valid q-range differs).
  - An all-ones column per head's V block accumulates softmax denominators
    in psum row 64 (AV cost is per-streamed-column, so output-partition
    padding is free). Causal: per k-block only the valid q-range is
    computed; the diagonal 128x128 block is masked by a DVE multiply with
    an upper-triangular tile.
  - Normalization per head: DVE copy of the denominator row,
    reciprocal_approx_fast, gpsimd partition-broadcast, DVE multiply.
  - Startup: weights/x are shipped HOST-PRE-PACKED so every big DMA is
    partition-contiguous (8KB runs, ~380 GB/s; the naive rearranged loads
    have 1KB runs at ~2-3x less). DMAs issue in consumption order; wv/wo
    are chained behind the critical path via marker-copy WAW deps. The
    chunk-0 Q and K projections run as one interleaved d-major wave over 8
    single-bank PSUM slots so the PE tracks DMA arrivals; V follows on 4
    slots. Dummy matmuls bridge the pre-DMA window to warm the HAM
    clock-gate.
  - Pipeline: chunk j's attention interleaves later chunks' projections as
    PE filler, rebalanced so chunk 3 (ACT-bound) gets exactly the 24
    out-projection groups of chunks 0..2: chunk0 <- proj(1)+Q(2),
    chunk1 <- K(2),V(2),Q(3), chunk2 <- K(3),V(3), chunk3 <- outproj x24.
    Chunk 3's own out-projection is the tail, bridged by dummies.
"""

import os
from contextlib import ExitStack

import numpy as np

import concourse.bacc as bacc
import concourse.mybir as mybir
import concourse.tile as tile
from concourse.bass_utils import run_bass_kernel_spmd
from concourse.masks import make_upper_triangular

F32 = mybir.dt.float32
BF16 = mybir.dt.bfloat16
AF = mybir.ActivationFunctionType
ALU = mybir.AluOpType

B = 4
S = 2048
D = 1024
HD = 64
HG = 8  # heads per core
QC = HG * HD  # 512 local q/k/v columns
N_CORES = 8

_NC_CACHE = {}
LAST_RESULT = None  # BassKernelResults of the most recent kernel() call


def _build_nc(s: int = S, num_devices: int = N_CORES):
    P = 128
    NQ = s // 512
    NS = s // P
    ND = D // P
    NT = QC // P  # 4 head pairs
    VW = HD + 1  # 65: per-head V block width (64 cols + ones col)
    VPAD = 7 * VW + P  # 583: last head's lhsT slice must fit

    nc = bacc.Bacc("TRN2", target_bir_lowering=False, debug=False, num_devices=num_devices)

    # All big inputs host-pre-packed to [128, ...] partition-contiguous.
    xT_d = nc.dram_tensor("xTp", [P, ND * s], BF16, kind="ExternalInput").ap()
    wq_d = nc.dram_tensor("wqp", [P, ND * QC], BF16, kind="ExternalInput").ap()
    wk_d = nc.dram_tensor("wkp", [P, ND * QC], BF16, kind="ExternalInput").ap()
    wv_d = nc.dram_tensor("wvp", [P, ND * QC], BF16, kind="ExternalInput").ap()
    wo_d = nc.dram_tensor("wop", [P, NT * D], BF16, kind="ExternalInput").ap()
    bq_d = nc.dram_tensor("bq", [QC], F32, kind="ExternalInput").ap()
    bk_d = nc.dram_tensor("bk", [QC], F32, kind="ExternalInput").ap()
    bv_d = nc.dram_tensor("bv", [QC], F32, kind="ExternalInput").ap()
    bo_d = nc.dram_tensor("bo", [D], F32, kind="ExternalInput").ap()
    out_d = nc.dram_tensor("out", [s, D], F32, kind="ExternalOutput").ap()

    with tile.TileContext(nc) as tc:
        with ExitStack() as ctx:
            consts = ctx.enter_context(tc.tile_pool(name="consts", bufs=1))
            persist = ctx.enter_context(tc.tile_pool(name="persist", bufs=1))
            e2_pool = ctx.enter_context(tc.tile_pool(name="e2pool", bufs=4))
            n_pool = ctx.enter_context(tc.tile_pool(name="npool", bufs=4))
            b_pool = ctx.enter_context(tc.tile_pool(name="bpool", bufs=4))
            o_pool = ctx.enter_context(tc.tile_pool(name="opool", bufs=3))
            proj_psum = ctx.enter_context(tc.tile_pool(name="proj_ps", bufs=2, space="PSUM"))
            s_psum = ctx.enter_context(tc.tile_pool(name="s_ps", bufs=2, space="PSUM"))
            a_psum = ctx.enter_context(tc.tile_pool(name="a_ps", bufs=2, space="PSUM"))

            # ---- dummy-weight tile via memset: warmup needs no DMA/gpsimd ----
            dmy = consts.tile([P, P], BF16)
            nc.vector.memset(dmy[:], 0.0078125)

            def dummy(n=1):
                """Keep-warm matmuls into a rotating score-pool bank (the
                score banks are idle at startup and at the tail; proj banks
                are WAR-chained behind filler evacuations)."""
                kw = s_psum.tile([P, 2, 512], F32, tag="s", name="kw")
                for _ in range(n):
                    nc.tensor.matmul(
                        kw[:, 0, 0:P], lhsT=dmy[:], rhs=dmy[:], start=True, stop=True
                    )

            # bridge the DMA-load window with continuous PE activity so the
            # HAM clock gate trips to 8/8 and stays there (~214ns each)
            dummy(40)

            tri = consts.tile([P, P], F32)
            make_upper_triangular(nc, tri[:], val=1.0, diag=True)
            tri_b = consts.tile([P, P], BF16)
            nc.any.tensor_copy(tri_b[:], tri[:])
            tri_bc = tri_b[:].rearrange("(p) c -> p () c").broadcast_to([P, 2, P])

            bqc = consts.tile([P, NT], F32)
            bkc = consts.tile([P, NT], F32)
            bv1 = consts.tile([1, QC], F32)
            bo1 = consts.tile([1, D], F32)
            bvb = consts.tile([P, QC], F32)
            bob = consts.tile([P, D], F32)

            # ---- persistent SBUF tensors ----
            QT = persist.tile([P, NT, s], BF16)
            KT = persist.tile([P, NT, s], BF16)
            # per-chunk diag K, zero-padded; double-buffered by chunk parity
            # (chunk j+1's K filler evac must not collide with chunk j's reads)
            KDz = persist.tile([P, HG, 2, 512], BF16)
            V = persist.tile([P, NS, VPAD + 1], BF16)
            AT = persist.tile([P, NT, s], BF16)
            xT = persist.tile([P, ND, s], BF16)
            wq_sb = persist.tile([P, ND, QC], BF16)
            wk_sb = persist.tile([P, ND, QC], BF16)
            wv_sb = persist.tile([P, ND, QC], BF16)
            wo_sb = persist.tile([P, NT, D], BF16)

            # V pad/ones + KDz constant zero halves, on-chip
            nc.any.memset(V[:, :, 7 * VW + HD + 1 :], 0.0)
            nc.any.memset(
                V[:, :, 0 : HG * VW].rearrange("p s (h c) -> p s h c", c=VW)[:, :, :, HD : HD + 1],
                1.0,
            )
            nc.any.memset(
                KDz[64:128].rearrange("p (t two) pr c -> p t two pr c", two=2)[:, :, 0], 0.0
            )
            nc.any.memset(
                KDz[0:64].rearrange("p (t two) pr c -> p t two pr c", two=2)[:, :, 1], 0.0
            )

            # ---- big input DMAs (contiguous), in consumption order ----
            nc.sync.dma_start(wq_sb[:], wq_d)
            nc.sync.dma_start(wk_sb[:], wk_d)
            for d in range(ND):
                nc.sync.dma_start(xT[:, d, :], xT_d[:, d * s : (d + 1) * s])
            # small consts (needed only by evacuations, ~20us in)
            nc.sync.dma_start(bqc[:], bq_d.rearrange("(t p) -> p t", p=P))
            nc.sync.dma_start(bkc[:], bk_d.rearrange("(t p) -> p t", p=P))
            nc.sync.dma_start(bv1[:], bv_d[None, :])
            nc.sync.dma_start(bo1[:], bo_d[None, :])
            nc.gpsimd.partition_broadcast(bvb[:], bv1[0:1, :])
            nc.gpsimd.partition_broadcast(bob[:], bo1[0:1, :])
            # wv/wo deferred via marker-copy WAW deps: they'd otherwise share
            # HBM bandwidth with (and delay) the critical path above.
            nc.vector.tensor_copy(wv_sb[0:1, 0, 0:2], xT[0:1, 5, 0:2])
            nc.sync.dma_start(wv_sb[:, 0:4, :], wv_d[:, 0 : 4 * QC])
            nc.sync.dma_start(wv_sb[:, 4:8, :], wv_d[:, 4 * QC : 8 * QC])
            nc.vector.tensor_copy(wo_sb[0:1, 0, 0:2], wv_sb[0:1, 7, 0:2])
            nc.sync.dma_start(wo_sb[:], wo_d)

            # ---- evacuation helpers ----
            def evac_q(ps, t, j):
                js = slice(j * 512, (j + 1) * 512)
                nc.vector.tensor_scalar_add(QT[:, t, js], ps[:], bqc[:, t : t + 1])

            def evac_k(ps, t, j):
                js = slice(j * 512, (j + 1) * 512)
                nc.vector.tensor_scalar_add(KT[:, t, js], ps[:], bkc[:, t : t + 1])
                # diag copy for chunk j (zero halves are persistent)
                nc.vector.tensor_scalar_add(
                    KDz[0:64, 2 * t, j % 2, :], ps[0:64, :], bkc[0:64, t : t + 1]
                )
                nc.vector.tensor_scalar_add(
                    KDz[64:128, 2 * t + 1, j % 2, :], ps[64:128, :], bkc[64:128, t : t + 1]
                )

            def evac_v(ps, st):
                dst = V[:, st, 0 : HG * VW].rearrange("p (h c) -> p h c", c=VW)[:, :, 0:HD]
                src = ps.rearrange("p (h c) -> p h c", c=HD)
                bsrc = bvb.rearrange("p (h c) -> p h c", c=HD)
                nc.vector.tensor_tensor(dst, src, bsrc, ALU.add)

            # ---- startup: interleaved Q+K chunk-0 wave over 8 banks ----
            qslots = [
                proj_psum.tile([P, 512], F32, tag="pp", name="wv0"),
                proj_psum.tile([P, 512], F32, tag="pp", name="wv1"),
                a_psum.tile([P, 512], F32, tag="a", name="wv2"),
                a_psum.tile([P, 512], F32, tag="a", name="wv3"),
            ]
            ks0 = s_psum.tile([P, 2, 512], F32, tag="s", name="ks0")
            ks1 = s_psum.tile([P, 2, 512], F32, tag="s", name="ks1")
            kslots = [ks0[:, 0, :], ks0[:, 1, :], ks1[:, 0, :], ks1[:, 1, :]]
            for d in range(ND):
                for t in range(NT):
                    nc.tensor.matmul(
                        qslots[t][:],
                        lhsT=wq_sb[:, d, t * P : (t + 1) * P],
                        rhs=xT[:, d, 0:512],
                        start=(d == 0),
                        stop=(d == ND - 1),
                    )
                for t in range(NT):
                    nc.tensor.matmul(
                        kslots[t],
                        lhsT=wk_sb[:, d, t * P : (t + 1) * P],
                        rhs=xT[:, d, 0:512],
                        start=(d == 0),
                        stop=(d == ND - 1),
                        skip_group_check=True,
                    )
            for t in range(NT):
                evac_q(qslots[t], t, 0)
            for t in range(NT):
                evac_k(kslots[t], t, 0)

            # V s-tiles 0..3, d-major over 4 banks
            vslots = [
                proj_psum.tile([P, 512], F32, tag="pp", name="vs0"),
                proj_psum.tile([P, 512], F32, tag="pp", name="vs1"),
                a_psum.tile([P, 512], F32, tag="a", name="vs2"),
                a_psum.tile([P, 512], F32, tag="a", name="vs3"),
            ]
            for d in range(ND):
                for st in range(4):
                    nc.tensor.matmul(
                        vslots[st][:],
                        lhsT=xT[:, d, st * P : (st + 1) * P],
                        rhs=wv_sb[:, d, :],
                        start=(d == 0),
                        stop=(d == ND - 1),
                    )
            for st in range(4):
                evac_v(vslots[st], st)

            # ---- filler units ----
            def proj_group(j, g):
                """One psum-group of the j-chunk projections; g in 0..11."""
                js = slice(j * 512, (j + 1) * 512)
                kind, t = divmod(g, NT)
                ps = proj_psum.tile([P, 512], F32, tag="pp", name="pp")
                if kind == 0:  # Q
                    for d in range(ND):
                        nc.tensor.matmul(
                            ps[:],
                            lhsT=wq_sb[:, d, t * P : (t + 1) * P],
                            rhs=xT[:, d, js],
                            start=(d == 0),
                            stop=(d == ND - 1),
                        )
                    evac_q(ps, t, j)
                elif kind == 1:  # K
                    for d in range(ND):
                        nc.tensor.matmul(
                            ps[:],
                            lhsT=wk_sb[:, d, t * P : (t + 1) * P],
                            rhs=xT[:, d, js],
                            start=(d == 0),
                            stop=(d == ND - 1),
                        )
                    evac_k(ps, t, j)
                else:  # V s-tile 4j+t
                    st = 4 * j + t
                    for d in range(ND):
                        nc.tensor.matmul(
                            ps[:],
                            lhsT=xT[:, d, st * P : (st + 1) * P],
                            rhs=wv_sb[:, d, :],
                            start=(d == 0),
                            stop=(d == ND - 1),
                        )
                    evac_v(ps, st)

            def out_proj_group(j, g):
                st = 4 * j + g // 2
                oc = g % 2
                o_ps = proj_psum.tile([P, 512], F32, tag="pp", name="o_ps")
                for t2 in range(NT):
                    nc.tensor.matmul(
                        o_ps[:],
                        lhsT=AT[:, t2, st * P : (st + 1) * P],
                        rhs=wo_sb[:, t2, oc * 512 : (oc + 1) * 512],
                        start=(t2 == 0),
                        stop=(t2 == NT - 1),
                    )
                ot = o_pool.tile([P, 512], F32, name="ot")
                nc.vector.tensor_tensor(
                    ot[:], o_ps[:], bob[:, oc * 512 : (oc + 1) * 512], ALU.add
                )
                nc.sync.dma_start(
                    out_d[st * P : (st + 1) * P, oc * 512 : (oc + 1) * 512], ot[:]
                )

            # ---- attention pair-chunk ----
            def attn_pair(j, t, filler, f_lo, f_hi):
                """Heads (2t, 2t+1) on q-chunk j. filler[f_lo:f_hi] emitted in
                128-mode regions: one slot per tiled block + 2 diag slots."""
                nkb = 4 * j + 4
                ntb = 2 * j  # tiled (off-diagonal) 2-round blocks
                nslot = ntb + 3  # 1 leading + per-block + 2 diagonal
                A0 = a_psum.tile([P, 512], F32, tag="a", name="A0")
                A1 = a_psum.tile([P, 512], F32, tag="a", name="A1")
                jq = j * 512
                nfill = f_hi - f_lo
                slot = 0

                def fill_slot():
                    nonlocal slot
                    k0 = f_lo + (nfill * slot) // nslot
                    k1 = f_lo + (nfill * (slot + 1)) // nslot
                    for f in filler[k0:k1]:
                        f()
                    slot += 1

                def av_round(r, y0, Erhs0, Erhs1):
                    nc.tensor.matmul(
                        A0[:, y0:],
                        lhsT=V[:, r, (2 * t) * VW : (2 * t) * VW + P],
                        rhs=Erhs0,
                        start=(r == 0),
                        stop=(r == nkb - 1),
                    )
                    nc.tensor.matmul(
                        A1[:, y0:],
                        lhsT=V[:, r, (2 * t + 1) * VW : (2 * t + 1) * VW + P],
                        rhs=Erhs1,
                        start=(r == 0),
                        stop=(r == nkb - 1),
                    )

                # leading slot covers the previous pair's exp/norm tail
                fill_slot()

                # off-diagonal: 64-row-tiled score pairs, 2 rounds per
                # block, one 2-bank batched exp per round
                for blk in range(ntb):
                    rr = (2 * blk, 2 * blk + 1)
                    Es = []
                    for r in rr:
                        S2 = s_psum.tile([P, 2, 512], F32, tag="s", name="S2")
                        nc.tensor.matmul(
                            S2[:, 0, :],
                            lhsT=KT[0:64, t, r * P : (r + 1) * P],
                            rhs=QT[0:64, t, jq : jq + 512],
                            start=True,
                            stop=True,
                        )
                        nc.tensor.matmul(
                            S2[:, 1, :],
                            lhsT=KT[64:128, t, r * P : (r + 1) * P],
                            rhs=QT[64:128, t, jq : jq + 512],
                            start=True,
                            stop=True,
                        )
                        E2 = e2_pool.tile([P, 2, 512], BF16, tag="e", name="E2")
                        nc.scalar.activation(E2[:], S2[:], AF.Exp, scale=0.125)
                        Es.append((r, E2))
                    fill_slot()
                    for r, E2 in Es:
                        av_round(r, 0, E2[:, 0, :], E2[:, 1, :])

                # diagonal region: 4 un-tiled (128-contraction) rounds using
                # S4 quarter-pairs, per-round exp (valid q-range differs)
                for half in range(2):
                    Es = []
                    for q2 in range(2):
                        r = 4 * j + 2 * half + q2
                        y0 = P * (r - 4 * j)
                        S2 = s_psum.tile([P, 2, 512], F32, tag="s", name="S2d")
                        for hh in range(2):
                            nc.tensor.matmul(
                                S2[:, hh, y0:],
                                lhsT=KDz[:, 2 * t + hh, j % 2, y0 : y0 + P],
                                rhs=QT[:, t, jq + y0 : jq + 512],
                                start=True,
                                stop=True,
                            )
                        E2 = e2_pool.tile([P, 2, 512], BF16, tag="e", name="E2")
                        nc.scalar.activation(
                            E2[:, :, y0:],
                            S2[:, :, y0:],
                            AF.Exp,
                            scale=0.125,
                        )
                        # causal mask on the diag block, both heads at once
                        nc.vector.tensor_tensor(
                            E2[:, :, y0 : y0 + P],
                            E2[:, :, y0 : y0 + P],
                            tri_bc,
                            ALU.mult,
                        )
                        Es.append((r, y0, E2))
                    fill_slot()
                    for r, y0, E2 in Es:
                        av_round(r, y0, E2[:, 0, y0:], E2[:, 1, y0:])

                # softmax normalization for both heads
                for i, A in enumerate((A0, A1)):
                    sums = n_pool.tile([1, 512], F32, tag="sums", name="sums")
                    nc.vector.tensor_copy(sums[:], A[HD : HD + 1, :])
                    rec = n_pool.tile([1, 512], F32, tag="rec", name="rec")
                    nc.vector.reciprocal_approx_fast(rec[:], sums[:])
                    bc = b_pool.tile([HD, 512], F32, name="bc")
                    nc.gpsimd.partition_broadcast(bc[:], rec[0:1, :])
                    nc.vector.tensor_tensor(
                        AT[64 * i : 64 * i + HD, t, jq : jq + 512],
                        A[0:HD, :],
                        bc[:],
                        ALU.mult,
                    )

            # ---- main pipeline; filler rebalanced toward chunk 3 ----
            def P_(jj, g):
                return lambda: proj_group(jj, g)

            def O_(jj, g):
                return lambda: out_proj_group(jj, g)

            fillers = [
                [P_(1, g) for g in range(12)] + [P_(2, g) for g in range(4)],
                [P_(2, g) for g in range(4, 12)] + [P_(3, g) for g in range(4)],
                [P_(3, g) for g in range(4, 12)],
                [O_(jo, g) for jo in range(NQ - 1) for g in range(8)],
            ]
            for j in range(NQ):
                filler = fillers[j]
                nf = len(filler)
                for t in range(NT):
                    attn_pair(j, t, filler, (nf * t) // NT, (nf * (t + 1)) // NT)

            # tail: bridge the last normalization, then chunk-3 out-proj
            dummy(24)
            for g in range(8):
                out_proj_group(NQ - 1, g)

    nc.compile()

    return nc


def _get_nc():
    if "nc" not in _NC_CACHE:
        _NC_CACHE["nc"] = _build_nc()
    return _NC_CACHE["nc"]


def _pack(w, nd=8):
    """[nd*128, C] -> [128, nd*C] partition-contiguous."""
    ndp, c = w.shape
    p = ndp // nd
    return np.ascontiguousarray(w.reshape(nd, p, c).transpose(1, 0, 2).reshape(p, nd * c))


def make_in_maps(x, wq, bq, wk, bk, wv, bv, wo, bo, n_cores=N_CORES):
    import ml_dtypes

    bf = ml_dtypes.bfloat16
    x = np.asarray(x, np.float32).astype(bf)
    wq, wk, wv, wo = (np.asarray(a, np.float32).astype(bf) for a in (wq, wk, wv, wo))
    bq, bk, bv, bo = (np.asarray(a, np.float32) for a in (bq, bk, bv, bo))
    in_maps = []
    for c in range(n_cores):
        b, g = c // 2, c % 2
        cs = slice(g * QC, (g + 1) * QC)
        in_maps.append(
            {
                "xTp": _pack(np.ascontiguousarray(x[b].T)),
                "wqp": _pack(np.ascontiguousarray(wq[:, cs])),
                "wkp": _pack(np.ascontiguousarray(wk[:, cs])),
                "wvp": _pack(np.ascontiguousarray(wv[:, cs])),
                "wop": _pack(np.ascontiguousarray(wo[cs, :]), nd=4),
                "bq": np.ascontiguousarray(bq[cs]),
                "bk": np.ascontiguousarray(bk[cs]),
                "bv": np.ascontiguousarray(bv[cs]),
                "bo": bo if g == 0 else np.zeros_like(bo),
            }
        )
    return in_maps


def kernel(x, wq, bq, wk, bk, wv, bv, wo, bo):
    global LAST_RESULT
    in_maps = make_in_maps(x, wq, bq, wk, bk, wv, bv, wo, bo)
    nc = _get_nc()
    trace = os.environ.get("MHA_TRACE", "0") == "1"
    res = run_bass_kernel_spmd(nc, in_maps, core_ids=list(range(N_CORES)), trace=trace)
    LAST_RESULT = res

    out = np.empty((B, S, D), np.float32)
    for b in range(B):
        out[b] = res.results[2 * b]["out"] + res.results[2 * b + 1]["out"]
    return out


# revision 17
# speedup vs baseline: 1.3753x; 1.0107x over previous
"""Causal multi-head attention for TRN2, sharded across 8 NeuronCores.

Problem: x[4,2048,1024] -> 16-head causal self-attention (head_dim 64) with
QKV + output projections, fp32.

Sharding: core c -> batch b = c // 2, head-group g = c % 2 (heads g*8..g*8+7).
Per core: Q/K/V projections use the 512 weight columns of its head-group
(column-parallel); attention runs over its 8 heads; the output projection
uses the matching 512 rows of wo (row-parallel), so each core emits a
partial [2048,1024] output and the host sums the two partials per batch.
bo is added on the g==0 cores only (g==1 cores receive zeros).

Device design (per core; S=2048, D=1024, HD=64; matmul operands bf16, all
accumulation fp32 in PSUM):
  - Heads processed as PAIRS: head 2t in SBUF partitions 0:64, head 2t+1
    in 64:128 (QT/KT/AT tiles [128, 4, S]). Off-diagonal score matmuls are
    64-row TILED (tiles T0/T8 via base_partition 0/64): both heads' scores
    run CONCURRENTLY on the PE (measured 2.0x). Mode switches (64<->128)
    cost ~106ns, so tiled scores run in 2-round blocks ([S x4] 64-mode |
    [filler + AV x4] 128-mode) and the 4 short DIAGONAL rounds per
    pair-chunk run un-tiled (128-contraction, zero-padded via the
    double-buffered KDz tile) inside the 128-mode region.
  - All score rounds land in ONE persistent 4-bank PSUM tile S4
    [128,4,512] (subtile deps give per-quarter synchronization). A tiled
    block's 4 banks are consumed by ONE batched exp instruction
    [128,4,512] (amortizes the ~210ns per-instruction PSUM-read latency;
    ACT exp is the chunk-3 co-bottleneck). Diagonal rounds exp per-round
    (valid q-range differs).
  - An all-ones column per head's V block accumulates softmax denominators
    in psum row 64 (AV cost is per-streamed-column, so output-partition
    padding is free). Causal: per k-block only the valid q-range is
    computed; the diagonal 128x128 block is masked by a DVE multiply with
    an upper-triangular tile.
  - Normalization per head: DVE copy of the denominator row,
    reciprocal_approx_fast, gpsimd partition-broadcast, DVE multiply.
  - Startup: weights/x are shipped HOST-PRE-PACKED so every big DMA is
    partition-contiguous (8KB runs, ~380 GB/s; the naive rearranged loads
    have 1KB runs at ~2-3x less). DMAs issue in consumption order; wv/wo
    are chained behind the critical path via marker-copy WAW deps. The
    chunk-0 Q and K projections run as one interleaved d-major wave over 8
    single-bank PSUM slots so the PE tracks DMA arrivals; V follows on 4
    slots. Dummy matmuls bridge the pre-DMA window to warm the HAM
    clock-gate.
  - Pipeline: chunk j's attention interleaves later chunks' projections as
    PE filler, rebalanced so chunk 3 (ACT-bound) gets exactly the 24
    out-projection groups of chunks 0..2: chunk0 <- proj(1)+Q(2),
    chunk1 <- K(2),V(2),Q(3), chunk2 <- K(3),V(3), chunk3 <- outproj x24.
    Chunk 3's own out-projection is the tail, bridged by dummies.
"""

import os
from contextlib import ExitStack

import numpy as np

import concourse.bacc as bacc
import concourse.mybir as mybir
import concourse.tile as tile
from concourse.bass_utils import run_bass_kernel_spmd
from concourse.masks import make_upper_triangular

F32 = mybir.dt.float32
BF16 = mybir.dt.bfloat16
AF = mybir.ActivationFunctionType
ALU = mybir.AluOpType

B = 4
S = 2048
D = 1024
HD = 64
HG = 8  # heads per core
QC = HG * HD  # 512 local q/k/v columns
N_CORES = 8

_NC_CACHE = {}
LAST_RESULT = None  # BassKernelResults of the most recent kernel() call


def _build_nc(s: int = S, num_devices: int = N_CORES):
    P = 128
    NQ = s // 512
    NS = s // P
    ND = D // P
    NT = QC // P  # 4 head pairs
    VW = HD + 1  # 65: per-head V block width (64 cols + ones col)
    VPAD = 7 * VW + P  # 583: last head's lhsT slice must fit

    nc = bacc.Bacc("TRN2", target_bir_lowering=False, debug=False, num_devices=num_devices)

    # All big inputs host-pre-packed to [128, ...] partition-contiguous.
    xT_d = nc.dram_tensor("xTp", [P, ND * s], BF16, kind="ExternalInput").ap()
    wq_d = nc.dram_tensor("wqp", [P, ND * QC], BF16, kind="ExternalInput").ap()
    wk_d = nc.dram_tensor("wkp", [P, ND * QC], BF16, kind="ExternalInput").ap()
    wv_d = nc.dram_tensor("wvp", [P, ND * QC], BF16, kind="ExternalInput").ap()
    wo_d = nc.dram_tensor("wop", [P, NT * D], BF16, kind="ExternalInput").ap()
    bq_d = nc.dram_tensor("bq", [QC], F32, kind="ExternalInput").ap()
    bk_d = nc.dram_tensor("bk", [QC], F32, kind="ExternalInput").ap()
    bv_d = nc.dram_tensor("bv", [QC], F32, kind="ExternalInput").ap()
    bo_d = nc.dram_tensor("bo", [D], F32, kind="ExternalInput").ap()
    out_d = nc.dram_tensor("out", [s, D], F32, kind="ExternalOutput").ap()

    with tile.TileContext(nc) as tc:
        with ExitStack() as ctx:
            consts = ctx.enter_context(tc.tile_pool(name="consts", bufs=1))
            persist = ctx.enter_context(tc.tile_pool(name="persist", bufs=1))
            e2_pool = ctx.enter_context(tc.tile_pool(name="e2pool", bufs=4))
            n_pool = ctx.enter_context(tc.tile_pool(name="npool", bufs=4))
            b_pool = ctx.enter_context(tc.tile_pool(name="bpool", bufs=4))
            o_pool = ctx.enter_context(tc.tile_pool(name="opool", bufs=3))
            proj_psum = ctx.enter_context(tc.tile_pool(name="proj_ps", bufs=2, space="PSUM"))
            s_psum = ctx.enter_context(tc.tile_pool(name="s_ps", bufs=2, space="PSUM"))
            a_psum = ctx.enter_context(tc.tile_pool(name="a_ps", bufs=2, space="PSUM"))

            # ---- dummy-weight tile via memset: warmup needs no DMA/gpsimd ----
            dmy = consts.tile([P, 512], BF16)
            nc.vector.memset(dmy[:], 0.0078125)

            def dummy(n=1, w=P):
                """Keep-warm matmuls into a rotating score-pool bank (the
                score banks are idle at startup and at the tail; proj banks
                are WAR-chained behind filler evacuations)."""
                kw = s_psum.tile([P, 2, 512], F32, tag="s", name="kw")
                for _ in range(n):
                    nc.tensor.matmul(
                        kw[:, 0, 0:w], lhsT=dmy[:, 0:P], rhs=dmy[:, 0:w],
                        start=True, stop=True,
                    )

            # bridge the DMA-load window with continuous PE activity so the
            # HAM clock gate trips to 8/8 (short mms) and STAYS there until
            # the first projection operands land (long mms)
            dummy(40)
            dummy(24, w=512)

            tri = consts.tile([P, P], F32)
            make_upper_triangular(nc, tri[:], val=1.0, diag=True)
            tri_b = consts.tile([P, P], BF16)
            nc.any.tensor_copy(tri_b[:], tri[:])
            tri_bc = tri_b[:].rearrange("(p) c -> p () c").broadcast_to([P, 2, P])

            bqc = consts.tile([P, NT], F32)
            bkc = consts.tile([P, NT], F32)
            bv1 = consts.tile([1, QC], F32)
            bo1 = consts.tile([1, D], F32)
            bvb = consts.tile([P, QC], F32)
            bob = consts.tile([P, D], F32)

            # ---- persistent SBUF tensors ----
            QT = persist.tile([P, NT, s], BF16)
            KT = persist.tile([P, NT, s], BF16)
            # per-chunk diag K, zero-padded; double-buffered by chunk parity
            # (chunk j+1's K filler evac must not collide with chunk j's reads)
            KDz = persist.tile([P, HG, 2, 512], BF16)
            V = persist.tile([P, NS, VPAD + 1], BF16)
            AT = persist.tile([P, NT, s], BF16)
            xT = persist.tile([P, ND, s], BF16)
            wq_sb = persist.tile([P, ND, QC], BF16)
            wk_sb = persist.tile([P, ND, QC], BF16)
            wv_sb = persist.tile([P, ND, QC], BF16)
            wo_sb = persist.tile([P, NT, D], BF16)

            # V pad/ones + KDz constant zero halves, on-chip
            nc.any.memset(V[:, :, 7 * VW + HD + 1 :], 0.0)
            nc.any.memset(
                V[:, :, 0 : HG * VW].rearrange("p s (h c) -> p s h c", c=VW)[:, :, :, HD : HD + 1],
                1.0,
            )
            nc.any.memset(
                KDz[64:128].rearrange("p (t two) pr c -> p t two pr c", two=2)[:, :, 0], 0.0
            )
            nc.any.memset(
                KDz[0:64].rearrange("p (t two) pr c -> p t two pr c", two=2)[:, :, 1], 0.0
            )

            # ---- big input DMAs (contiguous), in consumption order ----
            nc.sync.dma_start(wq_sb[:], wq_d)
            nc.sync.dma_start(wk_sb[:], wk_d)
            for d in range(ND):
                nc.sync.dma_start(xT[:, d, :], xT_d[:, d * s : (d + 1) * s])
            # small consts (needed only by evacuations, ~20us in)
            nc.sync.dma_start(bqc[:], bq_d.rearrange("(t p) -> p t", p=P))
            nc.sync.dma_start(bkc[:], bk_d.rearrange("(t p) -> p t", p=P))
            nc.sync.dma_start(bv1[:], bv_d[None, :])
            nc.sync.dma_start(bo1[:], bo_d[None, :])
            nc.gpsimd.partition_broadcast(bvb[:], bv1[0:1, :])
            nc.gpsimd.partition_broadcast(bob[:], bo1[0:1, :])
            # wv/wo deferred via marker-copy WAW deps: they'd otherwise share
            # HBM bandwidth with (and delay) the critical path above.
            nc.vector.tensor_copy(wv_sb[0:1, 0, 0:2], xT[0:1, 5, 0:2])
            nc.sync.dma_start(wv_sb[:, 0:4, :], wv_d[:, 0 : 4 * QC])
            nc.sync.dma_start(wv_sb[:, 4:8, :], wv_d[:, 4 * QC : 8 * QC])
            nc.vector.tensor_copy(wo_sb[0:1, 0, 0:2], wv_sb[0:1, 7, 0:2])
            nc.sync.dma_start(wo_sb[:], wo_d)

            # ---- evacuation helpers ----
            def evac_q(ps, t, j):
                js = slice(j * 512, (j + 1) * 512)
                nc.vector.tensor_scalar_add(QT[:, t, js], ps[:], bqc[:, t : t + 1])

            def evac_k(ps, t, j):
                js = slice(j * 512, (j + 1) * 512)
                nc.vector.tensor_scalar_add(KT[:, t, js], ps[:], bkc[:, t : t + 1])
                # diag copy for chunk j (zero halves are persistent)
                nc.vector.tensor_scalar_add(
                    KDz[0:64, 2 * t, j % 2, :], ps[0:64, :], bkc[0:64, t : t + 1]
                )
                nc.vector.tensor_scalar_add(
                    KDz[64:128, 2 * t + 1, j % 2, :], ps[64:128, :], bkc[64:128, t : t + 1]
                )

            def evac_v(ps, st):
                dst = V[:, st, 0 : HG * VW].rearrange("p (h c) -> p h c", c=VW)[:, :, 0:HD]
                src = ps.rearrange("p (h c) -> p h c", c=HD)
                bsrc = bvb.rearrange("p (h c) -> p h c", c=HD)
                nc.vector.tensor_tensor(dst, src, bsrc, ALU.add)

            # ---- startup: interleaved Q+K chunk-0 wave over 8 banks ----
            qslots = [
                proj_psum.tile([P, 512], F32, tag="pp", name="wv0"),
                proj_psum.tile([P, 512], F32, tag="pp", name="wv1"),
                a_psum.tile([P, 512], F32, tag="a", name="wv2"),
                a_psum.tile([P, 512], F32, tag="a", name="wv3"),
            ]
            ks0 = s_psum.tile([P, 2, 512], F32, tag="s", name="ks0")
            ks1 = s_psum.tile([P, 2, 512], F32, tag="s", name="ks1")
            kslots = [ks0[:, 0, :], ks0[:, 1, :], ks1[:, 0, :], ks1[:, 1, :]]
            for d in range(ND):
                for t in range(NT):
                    nc.tensor.matmul(
                        qslots[t][:],
                        lhsT=wq_sb[:, d, t * P : (t + 1) * P],
                        rhs=xT[:, d, 0:512],
                        start=(d == 0),
                        stop=(d == ND - 1),
                    )
                for t in range(NT):
                    nc.tensor.matmul(
                        kslots[t],
                        lhsT=wk_sb[:, d, t * P : (t + 1) * P],
                        rhs=xT[:, d, 0:512],
                        start=(d == 0),
                        stop=(d == ND - 1),
                        skip_group_check=True,
                    )
            for t in range(NT):
                evac_q(qslots[t], t, 0)
            for t in range(NT):
                evac_k(kslots[t], t, 0)

            # V s-tiles 0..3, d-major over 4 banks
            vslots = [
                proj_psum.tile([P, 512], F32, tag="pp", name="vs0"),
                proj_psum.tile([P, 512], F32, tag="pp", name="vs1"),
                a_psum.tile([P, 512], F32, tag="a", name="vs2"),
                a_psum.tile([P, 512], F32, tag="a", name="vs3"),
            ]
            for d in range(ND):
                for st in range(4):
                    nc.tensor.matmul(
                        vslots[st][:],
                        lhsT=xT[:, d, st * P : (st + 1) * P],
                        rhs=wv_sb[:, d, :],
                        start=(d == 0),
                        stop=(d == ND - 1),
                    )
            for st in range(4):
                evac_v(vslots[st], st)

            # ---- filler units ----
            def proj_group(j, g):
                """One psum-group of the j-chunk projections; g in 0..11."""
                js = slice(j * 512, (j + 1) * 512)
                kind, t = divmod(g, NT)
                ps = proj_psum.tile([P, 512], F32, tag="pp", name="pp")
                if kind == 0:  # Q
                    for d in range(ND):
                        nc.tensor.matmul(
                            ps[:],
                            lhsT=wq_sb[:, d, t * P : (t + 1) * P],
                            rhs=xT[:, d, js],
                            start=(d == 0),
                            stop=(d == ND - 1),
                        )
                    evac_q(ps, t, j)
                elif kind == 1:  # K
                    for d in range(ND):
                        nc.tensor.matmul(
                            ps[:],
                            lhsT=wk_sb[:, d, t * P : (t + 1) * P],
                            rhs=xT[:, d, js],
                            start=(d == 0),
                            stop=(d == ND - 1),
                        )
                    evac_k(ps, t, j)
                else:  # V s-tile 4j+t
                    st = 4 * j + t
                    for d in range(ND):
                        nc.tensor.matmul(
                            ps[:],
                            lhsT=xT[:, d, st * P : (st + 1) * P],
                            rhs=wv_sb[:, d, :],
                            start=(d == 0),
                            stop=(d == ND - 1),
                        )
                    evac_v(ps, st)

            def out_proj_group(j, g):
                st = 4 * j + g // 2
                oc = g % 2
                o_ps = proj_psum.tile([P, 512], F32, tag="pp", name="o_ps")
                for t2 in range(NT):
                    nc.tensor.matmul(
                        o_ps[:],
                        lhsT=AT[:, t2, st * P : (st + 1) * P],
                        rhs=wo_sb[:, t2, oc * 512 : (oc + 1) * 512],
                        start=(t2 == 0),
                        stop=(t2 == NT - 1),
                    )
                ot = o_pool.tile([P, 512], F32, name="ot")
                nc.vector.tensor_tensor(
                    ot[:], o_ps[:], bob[:, oc * 512 : (oc + 1) * 512], ALU.add
                )
                nc.sync.dma_start(
                    out_d[st * P : (st + 1) * P, oc * 512 : (oc + 1) * 512], ot[:]
                )

            def proj_pieces(j, g):
                """proj_group split into 4 pieces of 2 accumulating mms."""
                js = slice(j * 512, (j + 1) * 512)
                kind, t = divmod(g, NT)
                state = {}

                def piece(d0):
                    def run():
                        if d0 == 0:
                            state["ps"] = proj_psum.tile(
                                [P, 512], F32, tag="pp", name="pp"
                            )
                        ps = state["ps"]
                        for d in (d0, d0 + 1):
                            if kind == 2:
                                st = 4 * j + t
                                nc.tensor.matmul(
                                    ps[:],
                                    lhsT=xT[:, d, st * P : (st + 1) * P],
                                    rhs=wv_sb[:, d, :],
                                    start=(d == 0),
                                    stop=(d == ND - 1),
                                )
                            else:
                                w_sb = wq_sb if kind == 0 else wk_sb
                                nc.tensor.matmul(
                                    ps[:],
                                    lhsT=w_sb[:, d, t * P : (t + 1) * P],
                                    rhs=xT[:, d, js],
                                    start=(d == 0),
                                    stop=(d == ND - 1),
                                )
                        if d0 == ND - 2:
                            if kind == 0:
                                evac_q(ps, t, j)
                            elif kind == 1:
                                evac_k(ps, t, j)
                            else:
                                evac_v(ps, 4 * j + t)

                    return run

                return [piece(d0) for d0 in range(0, ND, 2)]

            def out_proj_pieces(j, g):
                """out_proj_group split into 2 pieces of 2 mms."""
                st = 4 * j + g // 2
                oc = g % 2
                state = {}

                def piece(t0):
                    def run():
                        if t0 == 0:
                            state["o_ps"] = proj_psum.tile(
                                [P, 512], F32, tag="pp", name="o_ps"
                            )
                        o_ps = state["o_ps"]
                        for t2 in (t0, t0 + 1):
                            nc.tensor.matmul(
                                o_ps[:],
                                lhsT=AT[:, t2, st * P : (st + 1) * P],
                                rhs=wo_sb[:, t2, oc * 512 : (oc + 1) * 512],
                                start=(t2 == 0),
                                stop=(t2 == NT - 1),
                            )
                        if t0 == NT - 2:
                            ot = o_pool.tile([P, 512], F32, name="ot")
                            nc.vector.tensor_tensor(
                                ot[:], o_ps[:], bob[:, oc * 512 : (oc + 1) * 512],
                                ALU.add,
                            )
                            nc.sync.dma_start(
                                out_d[st * P : (st + 1) * P, oc * 512 : (oc + 1) * 512],
                                ot[:],
                            )

                    return run

                return [piece(t0) for t0 in range(0, NT, 2)]

            # ---- attention pair-chunk ----
            def attn_pair(j, t, filler, f_lo, f_hi):
                """Heads (2t, 2t+1) on q-chunk j. filler[f_lo:f_hi] emitted in
                128-mode regions: one slot per tiled block + 2 diag slots."""
                nkb = 4 * j + 4
                ntb = 2 * j  # tiled (off-diagonal) 2-round blocks
                nslot = ntb + 3  # 1 leading + per-block + 2 diagonal
                A0 = a_psum.tile([P, 512], F32, tag="a", name="A0")
                A1 = a_psum.tile([P, 512], F32, tag="a", name="A1")
                jq = j * 512
                nfill = f_hi - f_lo
                slot = 0

                def fill_slot(av0=None, av1=None):
                    """Emit this slot's filler pieces; with av0/av1 given,
                    emit [half, av0, rest, av1] so each AV round's exp
                    latency is covered by filler work."""
                    nonlocal slot
                    k0 = f_lo + (nfill * slot) // nslot
                    k1 = f_lo + (nfill * (slot + 1)) // nslot
                    pieces = filler[k0:k1]
                    h = (len(pieces) + 1) // 2 if av0 is not None else len(pieces)
                    for f in pieces[:h]:
                        f()
                    if av0 is not None:
                        av0()
                    for f in pieces[h:]:
                        f()
                    if av1 is not None:
                        av1()
                    slot += 1

                def av_round(r, y0, Erhs0, Erhs1):
                    # 65-col lhsT: halves the weight-load time vs a 128 slice
                    nc.tensor.matmul(
                        A0[0:VW, y0:],
                        lhsT=V[:, r, (2 * t) * VW : (2 * t + 1) * VW],
                        rhs=Erhs0,
                        start=(r == 0),
                        stop=(r == nkb - 1),
                    )
                    nc.tensor.matmul(
                        A1[0:VW, y0:],
                        lhsT=V[:, r, (2 * t + 1) * VW : (2 * t + 2) * VW],
                        rhs=Erhs1,
                        start=(r == 0),
                        stop=(r == nkb - 1),
                    )

                # leading slot covers the previous pair's exp/norm tail
                fill_slot()

                # off-diagonal: 64-row-tiled score pairs, 2 rounds per
                # block, one 2-bank batched exp per round
                for blk in range(ntb):
                    rr = (2 * blk, 2 * blk + 1)
                    Es = []
                    for r in rr:
                        S2 = s_psum.tile([P, 2, 512], F32, tag="s", name="S2")
                        nc.tensor.matmul(
                            S2[:, 0, :],
                            lhsT=KT[0:64, t, r * P : (r + 1) * P],
                            rhs=QT[0:64, t, jq : jq + 512],
                            start=True,
                            stop=True,
                        )
                        nc.tensor.matmul(
                            S2[:, 1, :],
                            lhsT=KT[64:128, t, r * P : (r + 1) * P],
                            rhs=QT[64:128, t, jq : jq + 512],
                            start=True,
                            stop=True,
                        )
                        E2 = e2_pool.tile([P, 2, 512], BF16, tag="e", name="E2")
                        nc.scalar.activation(E2[:], S2[:], AF.Exp, scale=0.125)
                        Es.append((r, E2))
                    (r0, Ea), (r1, Eb) = Es
                    fill_slot(
                        av0=lambda: av_round(r0, 0, Ea[:, 0, :], Ea[:, 1, :]),
                        av1=lambda: av_round(r1, 0, Eb[:, 0, :], Eb[:, 1, :]),
                    )

                # diagonal region: 4 un-tiled (128-contraction) rounds using
                # S4 quarter-pairs, per-round exp (valid q-range differs)
                for half in range(2):
                    Es = []
                    for q2 in range(2):
                        r = 4 * j + 2 * half + q2
                        y0 = P * (r - 4 * j)
                        S2 = s_psum.tile([P, 2, 512], F32, tag="s", name="S2d")
                        for hh in range(2):
                            nc.tensor.matmul(
                                S2[:, hh, y0:],
                                lhsT=KDz[:, 2 * t + hh, j % 2, y0 : y0 + P],
                                rhs=QT[:, t, jq + y0 : jq + 512],
                                start=True,
                                stop=True,
                            )
                        E2 = e2_pool.tile([P, 2, 512], BF16, tag="e", name="E2")
                        nc.scalar.activation(
                            E2[:, :, y0:],
                            S2[:, :, y0:],
                            AF.Exp,
                            scale=0.125,
                        )
                        # causal mask on the diag block, both heads at once
                        nc.vector.tensor_tensor(
                            E2[:, :, y0 : y0 + P],
                            E2[:, :, y0 : y0 + P],
                            tri_bc,
                            ALU.mult,
                        )
                        Es.append((r, y0, E2))
                    (r0, ya, Ea), (r1, yb, Eb) = Es
                    fill_slot(
                        av0=lambda: av_round(r0, ya, Ea[:, 0, ya:], Ea[:, 1, ya:]),
                        av1=lambda: av_round(r1, yb, Eb[:, 0, yb:], Eb[:, 1, yb:]),
                    )

                # softmax normalization, both heads' chains interleaved so
                # the DVE/gpsimd stages pipeline instead of running serially
                sums, recs, bcs = [], [], []
                for i, A in enumerate((A0, A1)):
                    sm = n_pool.tile([1, 512], F32, tag="sums", name="sums")
                    nc.vector.tensor_copy(sm[:], A[HD : HD + 1, :])
                    sums.append(sm)
                for i in range(2):
                    rec = n_pool.tile([1, 512], F32, tag="rec", name="rec")
                    nc.vector.reciprocal_approx_fast(rec[:], sums[i][:])
                    recs.append(rec)
                for i in range(2):
                    bc = b_pool.tile([HD, 512], F32, name="bc")
                    nc.gpsimd.partition_broadcast(bc[:], recs[i][0:1, :])
                    bcs.append(bc)
                for i, A in enumerate((A0, A1)):
                    nc.vector.tensor_tensor(
                        AT[64 * i : 64 * i + HD, t, jq : jq + 512],
                        A[0:HD, :],
                        bcs[i][:],
                        ALU.mult,
                    )

            # ---- main pipeline; filler rebalanced toward chunk 3 ----
            fillers = [
                [pc for g in range(12) for pc in proj_pieces(1, g)]
                + [pc for g in range(4) for pc in proj_pieces(2, g)],
                [pc for g in range(4, 12) for pc in proj_pieces(2, g)]
                + [pc for g in range(4) for pc in proj_pieces(3, g)],
                [pc for g in range(4, 12) for pc in proj_pieces(3, g)],
                [pc for jo in range(NQ - 1) for g in range(8)
                 for pc in out_proj_pieces(jo, g)],
            ]
            for j in range(NQ):
                filler = fillers[j]
                nf = len(filler)
                for t in range(NT):
                    attn_pair(j, t, filler, (nf * t) // NT, (nf * (t + 1)) // NT)

            # tail: bridge the last normalization, then chunk-3 out-proj
            dummy(24)
            for g in range(8):
                out_proj_group(NQ - 1, g)

    nc.compile()

    return nc


def _get_nc():
    if "nc" not in _NC_CACHE:
        _NC_CACHE["nc"] = _build_nc()
    return _NC_CACHE["nc"]


def _pack(w, nd=8):
    """[nd*128, C] -> [128, nd*C] partition-contiguous."""
    ndp, c = w.shape
    p = ndp // nd
    return np.ascontiguousarray(w.reshape(nd, p, c).transpose(1, 0, 2).reshape(p, nd * c))


def make_in_maps(x, wq, bq, wk, bk, wv, bv, wo, bo, n_cores=N_CORES):
    import ml_dtypes

    bf = ml_dtypes.bfloat16
    x = np.asarray(x, np.float32).astype(bf)
    wq, wk, wv, wo = (np.asarray(a, np.float32).astype(bf) for a in (wq, wk, wv, wo))
    bq, bk, bv, bo = (np.asarray(a, np.float32) for a in (bq, bk, bv, bo))
    in_maps = []
    for c in range(n_cores):
        b, g = c // 2, c % 2
        cs = slice(g * QC, (g + 1) * QC)
        in_maps.append(
            {
                "xTp": _pack(np.ascontiguousarray(x[b].T)),
                "wqp": _pack(np.ascontiguousarray(wq[:, cs])),
                "wkp": _pack(np.ascontiguousarray(wk[:, cs])),
                "wvp": _pack(np.ascontiguousarray(wv[:, cs])),
                "wop": _pack(np.ascontiguousarray(wo[cs, :]), nd=4),
                "bq": np.ascontiguousarray(bq[cs]),
                "bk": np.ascontiguousarray(bk[cs]),
                "bv": np.ascontiguousarray(bv[cs]),
                "bo": bo if g == 0 else np.zeros_like(bo),
            }
        )
    return in_maps


def kernel(x, wq, bq, wk, bk, wv, bv, wo, bo):
    global LAST_RESULT
    in_maps = make_in_maps(x, wq, bq, wk, bk, wv, bv, wo, bo)
    nc = _get_nc()
    trace = os.environ.get("MHA_TRACE", "0") == "1"
    res = run_bass_kernel_spmd(nc, in_maps, core_ids=list(range(N_CORES)), trace=trace)
    LAST_RESULT = res

    out = np.empty((B, S, D), np.float32)
    for b in range(B):
        out[b] = res.results[2 * b]["out"] + res.results[2 * b + 1]["out"]
    return out
